# revision 10
# baseline (speedup 1.0000x reference)
"""Trainium2 Bass kernel for NeuromodulatedHolographicBrain.

Math (reference):
    r_gate  = sigmoid(x @ router_w.T + router_b)            # [B, 64]
    mask    = repeat(r_gate, 64, axis=1)                    # [B, H]
    sensory = (x @ W + bW) * mask                           # W from COO edges
    rec     = h_prev @ R + bR
    target  = tanh(sensory + rec)
    h_new   = h_prev + gate * (target - h_prev) * (DT/tau)
    pred    = h_new @ P + bP
    return (h_new, pred)

Strategy: densify the 1%-sparse edge-list weights on the host, then run
dense bf16 matmuls (fp32 PSUM accumulation) on the PE array. Hidden dim
(4096) is column-sharded across 8 cores (512 cols each): each core reads
x^T and h_prev^T in full, its own W/R column slabs and P row slab,
computes its h_new^T shard and a full [B, H] pred partial (contraction
over its h_new shard); the host sums the 8 partials. Activations live in
transposed layout [features(partitions), batch(free)] so no device
transposes are needed; the pred phase flips to batch-major output with
h_new^T blocks as the stationary operand so each loaded stationary is
reused by 8 matmuls (LDWEIGHTS dedup). All bulk DMAs are ~1-2 MiB with
>=8KB per-partition contiguous runs for ~340 GB/s.
"""

import numpy as np

B = 512
IN = 2048
H = 4096
SH = 512          # hidden cols per core
NCORES = 8
KA = IN // 128    # 16  K-tiles for x contraction
KC = H // 128     # 32  K-tiles for h contraction
M = SH // 128     # 4   m-tiles per shard
HC = H // B       # 8   hid chunks in pred phase
GA = KA // 4      # 4   awr groups (4 K-tiles per DMA)
GC = KC // 4      # 8   hr groups
DT = 0.1
RB = 64           # router blocks
RSH = RB // NCORES  # 8 router blocks per core
AW = B + SH + RSH   # 1032 awr row width
HW = B + SH         # 1024 hr row width

_prog = None
MMDT = "bfloat16"   # matmul operand dtype: "bfloat16" or "float32r"


def _dedup_ldweights(nc):
    """Drop InstLdweights that reload the exact weights already resident
    in the PE array (same memref/offset/pattern as the previous LDW with
    no other LDW in between). Carries any sync waits onto the next kept
    instruction. Run before _legalize_waits."""
    import concourse.mybir as mybir
    removed = 0
    for f in nc.m.functions:
        for blk in f.blocks:
            out = []
            last_key = None
            pend_w, pend_u = [], []
            for ins in blk.instructions:
                nm = type(ins).__name__
                if nm == 'InstLdweights':
                    a = ins.ins[0]
                    key = (a.memref, a.offset, str(a.ap), str(a.dtype))
                    if key == last_key:
                        if ins.sync_info is not None:
                            pend_w.extend(ins.sync_info.on_wait)
                            pend_u.extend(ins.sync_info.on_update)
                        removed += 1
                        continue
                    last_key = key
                elif nm == 'InstMatmult':
                    pass          # keeps the loaded weights
                elif nm in ('InstNoOp', 'InstEventSemaphore'):
                    pass          # no effect on PE array state
                else:
                    last_key = None
                if pend_w or pend_u:
                    if ins.sync_info is None:
                        ins.sync_info = mybir.SyncInfo(on_wait=[], on_update=[])
                    ins.sync_info.on_wait = pend_w + list(ins.sync_info.on_wait)
                    ins.sync_info.on_update = (list(ins.sync_info.on_update)
                                               + pend_u)
                    pend_w, pend_u = [], []
                out.append(ins)
            blk.instructions[:] = out
    return removed


def _legalize_waits(nc, mybir, max_waits=1):
    """Split multi-wait instructions into single-wait NoOps.

    The walrus build here rejects >1 piggybacked sync wait per instruction
    (seen on S3_LW-lowered matmuls and Drains). Run after TileContext
    exit, before nc.finalize()."""
    ctr = 0
    n_split = 0
    for f in nc.m.functions:
        for blk in f.blocks:
            out = []
            for ins in blk.instructions:
                si = ins.sync_info
                if si is not None and len(si.on_wait) > max_waits:
                    waits = list(si.on_wait)
                    extra, keep = waits[:-max_waits], waits[-max_waits:]
                    for w in extra:
                        ctr += 1
                        nop = mybir.InstNoOp(name=f"waitnop-{ctr}")
                        nop.engine = ins.engine
                        nop.sync_info = mybir.SyncInfo(on_wait=[w], on_update=[])
                        out.append(nop)
                        n_split += 1
                    si.on_wait = keep
                out.append(ins)
            blk.instructions[:] = out
    return n_split


def _build_program():
    import concourse.bass as bass
    import concourse.mybir as mybir
    import concourse.tile as tile

    f32 = mybir.dt.float32
    mdt = mybir.dt.bfloat16 if MMDT == "bfloat16" else mybir.dt.float32r
    Alu = mybir.AluOpType
    Act = mybir.ActivationFunctionType

    nc = bass.Bass()

    # partition-major group layouts: one ~1-2MB DMA per tensor/group
    awr_d = nc.dram_tensor("awr", [GA, 128, 4 * AW], mdt, kind="ExternalInput")
    hr_d = nc.dram_tensor("hr", [GC, 128, 4 * HW], mdt, kind="ExternalInput")
    p_d = nc.dram_tensor("p", [M, 128, H], mdt, kind="ExternalInput")
    hpa_d = nc.dram_tensor("hpa", [128, M, B], f32, kind="ExternalInput")
    g_d = nc.dram_tensor("g", [128, M, B], f32, kind="ExternalInput")
    eb_d = nc.dram_tensor("eb", [RSH, B], mdt, kind="ExternalInput")
    bias_d = nc.dram_tensor("bias", [128, 2 * M + 1], f32, kind="ExternalInput")
    hn_d = nc.dram_tensor("hn", [128, M, B], f32, kind="ExternalOutput")
    pp_d = nc.dram_tensor("pp", [M, 128, HC, B], f32, kind="ExternalOutput")

    with tile.TileContext(nc) as tc:
        with (
            tc.tile_pool(name="consts", bufs=1) as consts,
            tc.tile_pool(name="astream", bufs=2) as astream,
            tc.tile_pool(name="cstream", bufs=2) as cstream,
            tc.tile_pool(name="pres", bufs=1) as pres,
            tc.tile_pool(name="sens", bufs=1) as senspool,
            tc.tile_pool(name="recs", bufs=1) as recpool,
            tc.tile_pool(name="hn", bufs=1) as hnpool,
            tc.tile_pool(name="tmp", bufs=2) as tmppool,
            tc.tile_pool(name="outb", bufs=2) as outpool,
        ):
            with (
                tc.tile_pool(name="acc", bufs=4, space="PSUM") as acc_pool,
                tc.tile_pool(name="psb", bufs=2, space="PSUM") as psb_pool,
                tc.tile_pool(name="psr", bufs=1, space="PSUM") as psr_pool,
            ):
                # ---- small constants ----
                eb_t = consts.tile([RSH, B], mdt, tag="eb")
                nc.sync.dma_start(eb_t[:], eb_d[:])
                bias_t = consts.tile([128, 2 * M + 1], f32, tag="bias")
                nc.sync.dma_start(bias_t[:], bias_d[:])

                # ---- phase C: rec accumulation over h_prev K-tiles (runs
                # first: drains via plain copies so sensory overlaps) ----
                rec_ps = [acc_pool.tile([128, B], f32, tag="acc",
                                        name=f"rec_ps{i}") for i in range(M)]
                p_t = pres.tile([128, M, H], mdt, tag="p")
                hpa_t = consts.tile([128, M, B], f32, tag="hpa")
                g_t = consts.tile([128, M, B], f32, tag="g")
                for c in range(GC):
                    c_t = cstream.tile([128, 4, HW], mdt, tag="hr")
                    nc.sync.dma_start(c_t[:], hr_d[c])
                    for j in range(4):
                        k = 4 * c + j
                        ht = c_t[:, j, 0:B]
                        for m in range(M):
                            nc.tensor.matmul(
                                rec_ps[m][:],
                                c_t[:, j, B + 128 * m:B + 128 * (m + 1)],
                                ht, start=(k == 0), stop=(k == KC - 1))
                    # spread the P-slab load (4x1MB) across phase C
                    if c % 2 == 0:
                        nc.sync.dma_start(p_t[:, c // 2, :], p_d[c // 2])

                rec_sb = []
                for m in range(M):
                    r_sb = recpool.tile([128, B], f32, tag=f"rec{m}",
                                        name=f"rec_sb{m}")
                    nc.scalar.activation(r_sb[:], rec_ps[m][:], Act.Copy)
                    rec_sb.append(r_sb)

                # ---- phase A: router + sensory over x K-tiles ----
                rg_ps = psr_pool.tile([RSH, B], f32, tag="rg")
                s_ps = [acc_pool.tile([128, B], f32, tag="acc",
                                      name=f"s_ps{i}") for i in range(M)]
                for c in range(GA):
                    a_t = astream.tile([128, 4, AW], mdt, tag="awr")
                    nc.sync.dma_start(a_t[:], awr_d[c])
                    for j in range(4):
                        k = 4 * c + j
                        xt = a_t[:, j, 0:B]
                        nc.tensor.matmul(rg_ps[:],
                                         a_t[:, j, B + SH:B + SH + RSH], xt,
                                         start=(k == 0), stop=(k == KA - 1))
                        for m in range(M):
                            nc.tensor.matmul(
                                s_ps[m][:],
                                a_t[:, j, B + 128 * m:B + 128 * (m + 1)],
                                xt, start=(k == 0), stop=(k == KA - 1))
                    # update-path constants, needed from phase D
                    if c == 1:
                        nc.sync.dma_start(hpa_t[:], hpa_d[:])
                    elif c == 2:
                        nc.sync.dma_start(g_t[:], g_d[:])

                # ---- phase B: sigmoid -> mask expand -> sensory drain ----
                rg32 = tmppool.tile([RSH, B], f32, tag="rg32")
                nc.scalar.activation(rg32[:], rg_ps[:], Act.Sigmoid,
                                     bias=bias_t[0:RSH, 2 * M:2 * M + 1],
                                     scale=1.0)
                rg_r = tmppool.tile([RSH, B], mdt, tag="rgr")
                nc.vector.tensor_copy(rg_r[:], rg32[:])

                sens = []
                for m in range(M):
                    mask_ps = psb_pool.tile([128, B], f32, tag="mask",
                                            name=f"mask_ps{m}")
                    nc.tensor.matmul(mask_ps[:], eb_t[:, 128 * m:128 * (m + 1)],
                                     rg_r[:], start=True, stop=True)
                    mask_sb = tmppool.tile([128, B], f32, tag="masksb",
                                           name=f"mask_sb{m}")
                    nc.scalar.activation(mask_sb[:], mask_ps[:], Act.Copy)
                    s_sb = senspool.tile([128, B], f32, tag=f"sens{m}",
                                         name=f"sens_sb{m}")
                    # (x@W + bW) * mask
                    nc.vector.scalar_tensor_tensor(
                        s_sb[:], s_ps[m][:], bias_t[:, m:m + 1], mask_sb[:],
                        op0=Alu.add, op1=Alu.mult)
                    sens.append(s_sb)

                # ---- phase D: target, h_new = tgt*g + hp*(1-g) ----
                hn_sb = hnpool.tile([128, M, B], f32, tag="hnsb")
                hn_ts = []
                for m in range(M):
                    tmp = tmppool.tile([128, B], f32, tag="dtmp", name=f"tmp{m}")
                    # (rec + bR) + sens
                    nc.vector.scalar_tensor_tensor(
                        tmp[:], rec_sb[m][:], bias_t[:, M + m:M + m + 1],
                        sens[m][:], op0=Alu.add, op1=Alu.add)
                    tgt = tmppool.tile([128, B], f32, tag="dtgt", name=f"tgt{m}")
                    nc.scalar.activation(tgt[:], tmp[:], Act.Tanh)
                    e_sb = tmppool.tile([128, B], f32, tag="de", name=f"e{m}")
                    nc.vector.tensor_mul(e_sb[:], tgt[:], g_t[:, m, :])
                    nc.vector.tensor_add(hn_sb[:, m, :], e_sb[:], hpa_t[:, m, :])
                    hn_r = hnpool.tile([128, B], mdt, tag=f"hnr{m}",
                                       name=f"hn_r{m}")
                    nc.vector.tensor_copy(hn_r[:], hn_sb[:, m, :])
                    hn_ts.append(hn_r)
                nc.sync.dma_start(hn_d[:], hn_sb[:])

            # ---- phase E: pred partial, batch-major out ----
            # out[bt-block rows (batch), hid] = sum_kb hn^T[kb, bt]^T @ P[kb]
            # stationary hn block reused by 8 moving P chunks (LDW dedup)
            with tc.tile_pool(name="eps", bufs=8, space="PSUM") as eps:
                for bt in range(M):
                    pps = [eps.tile([128, B], f32, tag="ep",
                                    name=f"pp_{bt}_{hc}") for hc in range(HC)]
                    for kb in range(M):
                        for hc in range(HC):
                            nc.tensor.matmul(
                                pps[hc][:],
                                hn_ts[kb][:, 128 * bt:128 * (bt + 1)],
                                p_t[:, kb, B * hc:B * (hc + 1)],
                                start=(kb == 0), stop=(kb == M - 1),
                                skip_group_check=True)
                    pp_sb = outpool.tile([128, HC, B], f32, tag="ppsb",
                                         name=f"pp_sb{bt}")
                    for hc in range(HC):
                        nc.scalar.activation(pp_sb[:, hc, :], pps[hc][:],
                                             Act.Copy)
                    nc.sync.dma_start(pp_d[bt], pp_sb[:])

    _dedup_ldweights(nc)
    _legalize_waits(nc, mybir)
    nc.finalize()
    return nc


def _get_program():
    global _prog
    if _prog is None:
        _prog = _build_program()
    return _prog


def _densify(rows, cols, vals, n_rows, n_cols):
    flat = rows.astype(np.int64) * n_cols + cols.astype(np.int64)
    dense = np.bincount(flat, weights=vals.astype(np.float64),
                        minlength=n_rows * n_cols)
    return dense.astype(np.float32).reshape(n_rows, n_cols)


def _group_pmajor(tiles, width):
    """[K,128,width] -> [K/4, 128, 4*width] with 4 K-tiles contiguous
    per partition row (>=8KB runs per partition per DMA)."""
    k = tiles.shape[0]
    return np.ascontiguousarray(
        tiles.reshape(k // 4, 4, 128, width).transpose(0, 2, 1, 3).reshape(
            k // 4, 128, 4 * width))


def kernel(x, h_prev, gate, W_vals, W_bias, R_vals, R_bias, P_vals, P_bias,
           router_w, router_b, tau, W_rows, W_cols, R_rows, R_cols,
           P_rows, P_cols):
    from concourse.bass_utils import run_bass_kernel_spmd

    if MMDT == "bfloat16":
        import ml_dtypes
        mmdt = ml_dtypes.bfloat16
    else:
        mmdt = np.float32

    x = np.asarray(x, np.float32)
    h_prev = np.asarray(h_prev, np.float32)
    gate = np.asarray(gate, np.float32)

    Wd = _densify(np.asarray(W_rows), np.asarray(W_cols), np.asarray(W_vals), IN, H)
    Rd = _densify(np.asarray(R_rows), np.asarray(R_cols), np.asarray(R_vals), H, H)
    Pd = _densify(np.asarray(P_rows), np.asarray(P_cols), np.asarray(P_vals), H, H)

    XT = np.ascontiguousarray(x.T).reshape(KA, 128, B)
    HpT = np.ascontiguousarray(h_prev.T).reshape(KC, 128, B)
    rwT = np.ascontiguousarray(np.asarray(router_w, np.float32).T)  # [2048, 64]
    dt_tau = (DT / np.asarray(tau, np.float32))                     # [4096]
    gate1 = gate.reshape(B)

    ebmat = np.zeros((RSH, B), np.float32)
    for j in range(RSH):
        ebmat[j, 64 * j:64 * (j + 1)] = 1.0
    ebmat = ebmat.astype(mmdt)

    in_maps = []
    for c in range(NCORES):
        sh = slice(SH * c, SH * (c + 1))
        w_slab = np.ascontiguousarray(Wd[:, sh]).reshape(KA, 128, SH)
        r_slab = np.ascontiguousarray(Rd[:, sh]).reshape(KC, 128, SH)
        p_slab = np.ascontiguousarray(Pd[sh, :]).reshape(M, 128, H)
        rw_slab = np.ascontiguousarray(
            rwT[:, RSH * c:RSH * (c + 1)]).reshape(KA, 128, RSH)
        awr = np.concatenate([XT, w_slab, rw_slab], axis=2).astype(mmdt)
        hr = np.concatenate([HpT, r_slab], axis=2).astype(mmdt)
        g_full = np.outer(dt_tau[sh], gate1).astype(np.float32)     # [SH, B]
        hp_sh = np.ascontiguousarray(h_prev.T[sh])                  # [SH, B]
        hpa_full = (hp_sh - g_full * hp_sh).astype(np.float32)      # hp*(1-g)
        hpa = np.ascontiguousarray(
            hpa_full.reshape(M, 128, B).transpose(1, 0, 2))
        g = np.ascontiguousarray(
            g_full.reshape(M, 128, B).transpose(1, 0, 2))
        bias = np.zeros((128, 2 * M + 1), np.float32)
        bias[:, 0:M] = np.asarray(W_bias, np.float32)[sh].reshape(M, 128).T
        bias[:, M:2 * M] = np.asarray(R_bias, np.float32)[sh].reshape(M, 128).T
        bias[0:RSH, 2 * M] = np.asarray(router_b, np.float32)[RSH * c:RSH * (c + 1)]
        in_maps.append({
            "awr": _group_pmajor(awr, AW),
            "hr": _group_pmajor(hr, HW),
            "p": p_slab.astype(mmdt),
            "hpa": hpa,
            "g": g,
            "eb": ebmat,
            "bias": bias,
        })

    nc = _get_program()
    res = run_bass_kernel_spmd(nc, in_maps, list(range(NCORES)))

    h_new = np.empty((B, H), np.float32)
    pred = np.zeros((B, H), np.float64)
    for c in range(NCORES):
        sh = slice(SH * c, SH * (c + 1))
        # hn: [128, M, B] -> [M, 128, B] -> [SH, B] -> transpose
        h_new[:, sh] = res.results[c]["hn"].transpose(1, 0, 2).reshape(SH, B).T
        # pp: [M(bt), 128, HC, B] batch-major partial [B, H]
        pred += res.results[c]["pp"].reshape(M * 128, H)
    pred = pred.astype(np.float32) + np.asarray(P_bias, np.float32)
    return (h_new, pred)


# revision 12
# speedup vs baseline: 1.1121x; 1.1121x over previous
"""Trainium2 Bass kernel for NeuromodulatedHolographicBrain.

Math (reference):
    r_gate  = sigmoid(x @ router_w.T + router_b)            # [B, 64]
    mask    = repeat(r_gate, 64, axis=1)                    # [B, H]
    sensory = (x @ W + bW) * mask                           # W from COO edges
    rec     = h_prev @ R + bR
    target  = tanh(sensory + rec)
    h_new   = h_prev + gate * (target - h_prev) * (DT/tau)
    pred    = h_new @ P + bP
    return (h_new, pred)

Strategy: densify the 1%-sparse edge-list weights on the host, then run
dense bf16 matmuls (fp32 PSUM accumulation) on the PE array. Hidden dim
(4096) is column-sharded across 8 cores (512 cols each): each core reads
x^T and h_prev^T in full, its own W/R column slabs and P row slab,
computes its h_new^T shard and a full [B, H] pred partial (contraction
over its h_new shard); the host sums the 8 partials. Activations live in
transposed layout [features(partitions), batch(free)] so no device
transposes are needed. The sensory phase runs m-outer so each hidden
m-tile's mask/tanh/update chain overlaps the next tile's matmuls; the
pred phase is batch-major with h_new^T blocks stationary so kb 0..2
matmuls can start while the last update chain finishes. All bulk DMAs
are ~1-2 MiB with >=4KB per-partition contiguous runs for high DMA rate.
"""

import numpy as np

B = 512
IN = 2048
H = 4096
SH = 512          # hidden cols per core
NCORES = 8
KA = IN // 128    # 16  K-tiles for x contraction
KC = H // 128     # 32  K-tiles for h contraction
M = SH // 128     # 4   m-tiles per shard
HC = H // B       # 8   hid chunks in pred phase
GC = KC // 4      # 8   hr groups
DT = 0.1
RB = 64           # router blocks
RSH = RB // NCORES  # 8 router blocks per core
XW = B + RSH        # 520 xtr row width
HW = B + SH         # 1024 hr row width

_prog = None
MMDT = "bfloat16"   # matmul operand dtype: "bfloat16" or "float32r"


def _dedup_ldweights(nc):
    """Drop InstLdweights that reload the exact weights already resident
    in the PE array (same memref/offset/pattern as the previous LDW with
    no other LDW in between). Carries sync waits/updates onto the next
    kept instruction. Run before _legalize_waits."""
    import concourse.mybir as mybir
    removed = 0
    for f in nc.m.functions:
        for blk in f.blocks:
            out = []
            last_key = None
            pend_w, pend_u = [], []
            for ins in blk.instructions:
                nm = type(ins).__name__
                if nm == 'InstLdweights':
                    a = ins.ins[0]
                    key = (a.memref, a.offset, str(a.ap), str(a.dtype))
                    if key == last_key:
                        if ins.sync_info is not None:
                            pend_w.extend(ins.sync_info.on_wait)
                            pend_u.extend(ins.sync_info.on_update)
                        removed += 1
                        continue
                    last_key = key
                elif nm == 'InstMatmult':
                    pass          # keeps the loaded weights
                elif nm in ('InstNoOp', 'InstEventSemaphore'):
                    pass          # no effect on PE array state
                else:
                    last_key = None
                if pend_w or pend_u:
                    if ins.sync_info is None:
                        ins.sync_info = mybir.SyncInfo(on_wait=[], on_update=[])
                    ins.sync_info.on_wait = pend_w + list(ins.sync_info.on_wait)
                    ins.sync_info.on_update = (list(ins.sync_info.on_update)
                                               + pend_u)
                    pend_w, pend_u = [], []
                out.append(ins)
            blk.instructions[:] = out
    return removed


def _legalize_waits(nc, mybir, max_waits=1):
    """Split multi-wait instructions into single-wait NoOps.

    The walrus build here rejects >1 piggybacked sync wait per instruction
    (seen on S3_LW-lowered matmuls and Drains). Run after TileContext
    exit, before nc.finalize()."""
    ctr = 0
    n_split = 0
    for f in nc.m.functions:
        for blk in f.blocks:
            out = []
            for ins in blk.instructions:
                si = ins.sync_info
                if si is not None and len(si.on_wait) > max_waits:
                    waits = list(si.on_wait)
                    extra, keep = waits[:-max_waits], waits[-max_waits:]
                    for w in extra:
                        ctr += 1
                        nop = mybir.InstNoOp(name=f"waitnop-{ctr}")
                        nop.engine = ins.engine
                        nop.sync_info = mybir.SyncInfo(on_wait=[w], on_update=[])
                        out.append(nop)
                        n_split += 1
                    si.on_wait = keep
                out.append(ins)
            blk.instructions[:] = out
    return n_split


def _build_program():
    import concourse.bass as bass
    import concourse.mybir as mybir
    import concourse.tile as tile

    f32 = mybir.dt.float32
    mdt = mybir.dt.bfloat16 if MMDT == "bfloat16" else mybir.dt.float32r
    Alu = mybir.AluOpType
    Act = mybir.ActivationFunctionType

    nc = bass.Bass()

    # partition-major layouts: one ~1-2MB DMA per tensor/group
    xtr_d = nc.dram_tensor("xtr", [2, 128, 8 * XW], mdt, kind="ExternalInput")
    w_d = nc.dram_tensor("w", [M, 128, KA * 128], mdt, kind="ExternalInput")
    hr_d = nc.dram_tensor("hr", [GC, 128, 4 * HW], mdt, kind="ExternalInput")
    p_d = nc.dram_tensor("p", [M, 128, H], mdt, kind="ExternalInput")
    hpa_d = nc.dram_tensor("hpa", [128, M, B], f32, kind="ExternalInput")
    g_d = nc.dram_tensor("g", [128, M, B], f32, kind="ExternalInput")
    eb_d = nc.dram_tensor("eb", [RSH, B], mdt, kind="ExternalInput")
    bias_d = nc.dram_tensor("bias", [128, 2 * M + 1], f32, kind="ExternalInput")
    hn_d = nc.dram_tensor("hn", [128, M, B], f32, kind="ExternalOutput")
    pp_d = nc.dram_tensor("pp", [M, 128, HC, B], f32, kind="ExternalOutput")

    with tile.TileContext(nc) as tc:
        with (
            tc.tile_pool(name="consts", bufs=1) as consts,
            tc.tile_pool(name="xres", bufs=1) as xres,
            tc.tile_pool(name="wstream", bufs=2) as wstream,
            tc.tile_pool(name="cstream", bufs=2) as cstream,
            tc.tile_pool(name="pres", bufs=1) as pres,
            tc.tile_pool(name="sens", bufs=1) as senspool,
            tc.tile_pool(name="recs", bufs=1) as recpool,
            tc.tile_pool(name="hn", bufs=1) as hnpool,
            tc.tile_pool(name="tmp", bufs=2) as tmppool,
            tc.tile_pool(name="outb", bufs=2) as outpool,
        ):
            with (
                tc.tile_pool(name="acc", bufs=4, space="PSUM") as acc_pool,
                tc.tile_pool(name="psb", bufs=2, space="PSUM") as psb_pool,
                tc.tile_pool(name="psr", bufs=1, space="PSUM") as psr_pool,
            ):
                # ---- small constants ----
                eb_t = consts.tile([RSH, B], mdt, tag="eb")
                nc.sync.dma_start(eb_t[:], eb_d[:])
                bias_t = consts.tile([128, 2 * M + 1], f32, tag="bias")
                nc.sync.dma_start(bias_t[:], bias_d[:])

                # ---- phase C: rec accumulation over h_prev K-tiles ----
                rec_ps = [acc_pool.tile([128, B], f32, tag="acc",
                                        name=f"rec_ps{i}") for i in range(M)]
                p_t = pres.tile([128, M, H], mdt, tag="p")
                xt_t = xres.tile([128, KA, XW], mdt, tag="xt")
                hpa_t = consts.tile([128, M, B], f32, tag="hpa")
                g_t = consts.tile([128, M, B], f32, tag="g")
                for c in range(GC):
                    c_t = cstream.tile([128, 4, HW], mdt, tag="hr")
                    nc.sync.dma_start(c_t[:], hr_d[c])
                    for j in range(4):
                        k = 4 * c + j
                        ht = c_t[:, j, 0:B]
                        for m in range(M):
                            nc.tensor.matmul(
                                rec_ps[m][:],
                                c_t[:, j, B + 128 * m:B + 128 * (m + 1)],
                                ht, start=(k == 0), stop=(k == KC - 1))
                    # interleave x / P loads into phase C DMA slack
                    if c == 1:
                        nc.sync.dma_start(xt_t[:, 0:8, :], xtr_d[0])
                    elif c == 3:
                        nc.sync.dma_start(xt_t[:, 8:16, :], xtr_d[1])
                    elif c == 5:
                        nc.sync.dma_start(p_t[:, 0, :], p_d[0])
                    elif c == 7:
                        nc.sync.dma_start(p_t[:, 1, :], p_d[1])

                rec_sb = []
                for m in range(M):
                    r_sb = recpool.tile([128, B], f32, tag=f"rec{m}",
                                        name=f"rec_sb{m}")
                    nc.scalar.activation(r_sb[:], rec_ps[m][:], Act.Copy)
                    rec_sb.append(r_sb)

                # ---- phase A: router, then sensory m-outer ----
                w_ts = []
                for m in range(M):
                    w_t = wstream.tile([128, KA * 128], mdt, tag="w",
                                       name=f"w_t{m}")
                    nc.sync.dma_start(w_t[:], w_d[m])
                    w_ts.append(w_t)

                rg_ps = psr_pool.tile([RSH, B], f32, tag="rg")
                for k in range(KA):
                    nc.tensor.matmul(rg_ps[:], xt_t[:, k, B:XW],
                                     xt_t[:, k, 0:B],
                                     start=(k == 0), stop=(k == KA - 1))
                rg32 = tmppool.tile([RSH, B], f32, tag="rg32")
                nc.scalar.activation(rg32[:], rg_ps[:], Act.Sigmoid,
                                     bias=bias_t[0:RSH, 2 * M:2 * M + 1],
                                     scale=1.0)
                rg_r = tmppool.tile([RSH, B], mdt, tag="rgr")
                nc.vector.tensor_copy(rg_r[:], rg32[:])

                # remaining bulk loads ride phase A's DMA slack
                nc.sync.dma_start(p_t[:, 2, :], p_d[2])
                nc.sync.dma_start(p_t[:, 3, :], p_d[3])
                nc.sync.dma_start(hpa_t[:], hpa_d[:])
                nc.sync.dma_start(g_t[:], g_d[:])

                hn_sb = hnpool.tile([128, M, B], f32, tag="hnsb")
                hn_ts = []
                for m in range(M):
                    s_ps = acc_pool.tile([128, B], f32, tag="acc",
                                         name=f"s_ps{m}")
                    for k in range(KA):
                        nc.tensor.matmul(
                            s_ps[:], w_ts[m][:, 128 * k:128 * (k + 1)],
                            xt_t[:, k, 0:B],
                            start=(k == 0), stop=(k == KA - 1))
                    mask_ps = psb_pool.tile([128, B], f32, tag="mask",
                                            name=f"mask_ps{m}")
                    nc.tensor.matmul(mask_ps[:], eb_t[:, 128 * m:128 * (m + 1)],
                                     rg_r[:], start=True, stop=True)
                    mask_sb = tmppool.tile([128, B], f32, tag="masksb",
                                           name=f"mask_sb{m}")
                    nc.scalar.activation(mask_sb[:], mask_ps[:], Act.Copy)
                    s_sb = senspool.tile([128, B], f32, tag=f"sens{m}",
                                         name=f"sens_sb{m}")
                    # (x@W + bW) * mask
                    nc.vector.scalar_tensor_tensor(
                        s_sb[:], s_ps[:], bias_t[:, m:m + 1], mask_sb[:],
                        op0=Alu.add, op1=Alu.mult)
                    # ---- phase D chain for this m (overlaps next m's MMs)
                    tmp = tmppool.tile([128, B], f32, tag="dtmp", name=f"tmp{m}")
                    nc.vector.scalar_tensor_tensor(
                        tmp[:], rec_sb[m][:], bias_t[:, M + m:M + m + 1],
                        s_sb[:], op0=Alu.add, op1=Alu.add)
                    tgt = tmppool.tile([128, B], f32, tag="dtgt", name=f"tgt{m}")
                    nc.scalar.activation(tgt[:], tmp[:], Act.Tanh)
                    e_sb = tmppool.tile([128, B], f32, tag="de", name=f"e{m}")
                    nc.vector.tensor_mul(e_sb[:], tgt[:], g_t[:, m, :])
                    nc.vector.tensor_add(hn_sb[:, m, :], e_sb[:],
                                         hpa_t[:, m, :])
                    hn_r = hnpool.tile([128, B], mdt, tag=f"hnr{m}",
                                       name=f"hn_r{m}")
                    nc.vector.tensor_copy(hn_r[:], hn_sb[:, m, :])
                    hn_ts.append(hn_r)
                nc.sync.dma_start(hn_d[:], hn_sb[:])

            # ---- phase E: pred partial, batch-major out ----
            # out[bt rows (batch), hid] = sum_kb hn^T[kb, bt]^T @ P[kb]
            # stationary hn block reused by 8 moving P chunks (LDW dedup);
            # kb 0..2 matmuls overlap the tail of the m=3 update chain
            with tc.tile_pool(name="eps", bufs=8, space="PSUM") as eps:
                for bt in range(M):
                    pps = [eps.tile([128, B], f32, tag="ep",
                                    name=f"pp_{bt}_{hc}") for hc in range(HC)]
                    for kb in range(M):
                        for hc in range(HC):
                            nc.tensor.matmul(
                                pps[hc][:],
                                hn_ts[kb][:, 128 * bt:128 * (bt + 1)],
                                p_t[:, kb, B * hc:B * (hc + 1)],
                                start=(kb == 0), stop=(kb == M - 1),
                                skip_group_check=True)
                    pp_sb = outpool.tile([128, HC, B], f32, tag="ppsb",
                                         name=f"pp_sb{bt}")
                    for hc in range(HC):
                        nc.scalar.activation(pp_sb[:, hc, :], pps[hc][:],
                                             Act.Copy)
                        if hc == HC // 2 - 1:
                            nc.sync.dma_start(pp_d[bt][:, 0:HC // 2, :],
                                              pp_sb[:, 0:HC // 2, :])
                    nc.sync.dma_start(pp_d[bt][:, HC // 2:HC, :],
                                      pp_sb[:, HC // 2:HC, :])

    _dedup_ldweights(nc)
    _legalize_waits(nc, mybir)
    nc.finalize()
    return nc


def _get_program():
    global _prog
    if _prog is None:
        _prog = _build_program()
    return _prog


def _densify(rows, cols, vals, n_rows, n_cols):
    flat = rows.astype(np.int64) * n_cols + cols.astype(np.int64)
    dense = np.bincount(flat, weights=vals.astype(np.float64),
                        minlength=n_rows * n_cols)
    return dense.astype(np.float32).reshape(n_rows, n_cols)


def _group_pmajor(tiles, width, group=4):
    """[K,128,width] -> [K/group, 128, group*width] with `group` K-tiles
    contiguous per partition row (large per-partition runs per DMA)."""
    k = tiles.shape[0]
    return np.ascontiguousarray(
        tiles.reshape(k // group, group, 128, width)
        .transpose(0, 2, 1, 3).reshape(k // group, 128, group * width))


def kernel(x, h_prev, gate, W_vals, W_bias, R_vals, R_bias, P_vals, P_bias,
           router_w, router_b, tau, W_rows, W_cols, R_rows, R_cols,
           P_rows, P_cols):
    from concourse.bass_utils import run_bass_kernel_spmd

    if MMDT == "bfloat16":
        import ml_dtypes
        mmdt = ml_dtypes.bfloat16
    else:
        mmdt = np.float32

    x = np.asarray(x, np.float32)
    h_prev = np.asarray(h_prev, np.float32)
    gate = np.asarray(gate, np.float32)

    Wd = _densify(np.asarray(W_rows), np.asarray(W_cols), np.asarray(W_vals), IN, H)
    Rd = _densify(np.asarray(R_rows), np.asarray(R_cols), np.asarray(R_vals), H, H)
    Pd = _densify(np.asarray(P_rows), np.asarray(P_cols), np.asarray(P_vals), H, H)

    XT = np.ascontiguousarray(x.T).reshape(KA, 128, B)
    HpT = np.ascontiguousarray(h_prev.T).reshape(KC, 128, B)
    rwT = np.ascontiguousarray(np.asarray(router_w, np.float32).T)  # [2048, 64]
    dt_tau = (DT / np.asarray(tau, np.float32))                     # [4096]
    gate1 = gate.reshape(B)

    ebmat = np.zeros((RSH, B), np.float32)
    for j in range(RSH):
        ebmat[j, 64 * j:64 * (j + 1)] = 1.0
    ebmat = ebmat.astype(mmdt)

    in_maps = []
    for c in range(NCORES):
        sh = slice(SH * c, SH * (c + 1))
        w_slab = np.ascontiguousarray(Wd[:, sh]).reshape(KA, 128, SH)
        # per-m partition-major: w2[m, p, 128*k+q] = W[128k+p, 128m+q]
        w2 = np.ascontiguousarray(
            w_slab.reshape(KA, 128, M, 128).transpose(2, 1, 0, 3)
            .reshape(M, 128, KA * 128)).astype(mmdt)
        r_slab = np.ascontiguousarray(Rd[:, sh]).reshape(KC, 128, SH)
        p_slab = np.ascontiguousarray(Pd[sh, :]).reshape(M, 128, H)
        rw_slab = np.ascontiguousarray(
            rwT[:, RSH * c:RSH * (c + 1)]).reshape(KA, 128, RSH)
        xtr = np.concatenate([XT, rw_slab], axis=2).astype(mmdt)    # [16,128,520]
        hr = np.concatenate([HpT, r_slab], axis=2).astype(mmdt)
        g_full = np.outer(dt_tau[sh], gate1).astype(np.float32)     # [SH, B]
        hp_sh = np.ascontiguousarray(h_prev.T[sh])                  # [SH, B]
        hpa_full = (hp_sh - g_full * hp_sh).astype(np.float32)      # hp*(1-g)
        hpa = np.ascontiguousarray(
            hpa_full.reshape(M, 128, B).transpose(1, 0, 2))
        g = np.ascontiguousarray(
            g_full.reshape(M, 128, B).transpose(1, 0, 2))
        bias = np.zeros((128, 2 * M + 1), np.float32)
        bias[:, 0:M] = np.asarray(W_bias, np.float32)[sh].reshape(M, 128).T
        bias[:, M:2 * M] = np.asarray(R_bias, np.float32)[sh].reshape(M, 128).T
        bias[0:RSH, 2 * M] = np.asarray(router_b, np.float32)[RSH * c:RSH * (c + 1)]
        in_maps.append({
            "xtr": _group_pmajor(xtr, XW, group=8),
            "w": w2,
            "hr": _group_pmajor(hr, HW, group=4),
            "p": p_slab.astype(mmdt),
            "hpa": hpa,
            "g": g,
            "eb": ebmat,
            "bias": bias,
        })

    nc = _get_program()
    res = run_bass_kernel_spmd(nc, in_maps, list(range(NCORES)))

    h_new = np.empty((B, H), np.float32)
    pred = np.zeros((B, H), np.float64)
    for c in range(NCORES):
        sh = slice(SH * c, SH * (c + 1))
        h_new[:, sh] = res.results[c]["hn"].transpose(1, 0, 2).reshape(SH, B).T
        pred += res.results[c]["pp"].reshape(M * 128, H)
    pred = pred.astype(np.float32) + np.asarray(P_bias, np.float32)
    return (h_new, pred)


# revision 13
# speedup vs baseline: 1.2054x; 1.0839x over previous
"""Trainium2 Bass kernel for NeuromodulatedHolographicBrain.

Math (reference):
    r_gate  = sigmoid(x @ router_w.T + router_b)            # [B, 64]
    mask    = repeat(r_gate, 64, axis=1)                    # [B, H]
    sensory = (x @ W + bW) * mask                           # W from COO edges
    rec     = h_prev @ R + bR
    target  = tanh(sensory + rec)
    h_new   = h_prev + gate * (target - h_prev) * (DT/tau)
    pred    = h_new @ P + bP
    return (h_new, pred)

Strategy: densify the 1%-sparse edge-list weights on the host, then run
dense bf16 matmuls (fp32 PSUM accumulation) on the PE array. Hidden dim
(4096) is column-sharded across 8 cores (512 cols each): each core reads
x^T and h_prev^T in full, its own W/R column slabs and P row slab,
computes its h_new^T shard and a full [B, H] pred partial (contraction
over its h_new shard); the host sums the 8 partials. Activations live in
transposed layout [features(partitions), batch(free)] so no device
transposes are needed. The sensory phase runs m-outer so each hidden
m-tile's mask/tanh/update chain overlaps the next tile's matmuls; the
pred phase is batch-major with h_new^T blocks stationary so kb 0..2
matmuls can start while the last update chain finishes. All bulk DMAs
are ~1-2 MiB with >=4KB per-partition contiguous runs for high DMA rate.
"""

import numpy as np

B = 512
IN = 2048
H = 4096
SH = 512          # hidden cols per core
NCORES = 8
KA = IN // 128    # 16  K-tiles for x contraction
KC = H // 128     # 32  K-tiles for h contraction
M = SH // 128     # 4   m-tiles per shard
HC = H // B       # 8   hid chunks in pred phase
GC = KC // 4      # 8   hr groups
DT = 0.1
RB = 64           # router blocks
RSH = RB // NCORES  # 8 router blocks per core
XW = B + RSH        # 520 xtr row width
HW = B + SH         # 1024 hr row width

_prog = None
MMDT = "bfloat16"   # matmul operand dtype: "bfloat16" or "float32r"


def _dedup_ldweights(nc):
    """Drop InstLdweights that reload the exact weights already resident
    in the PE array (same memref/offset/pattern as the previous LDW with
    no other LDW in between). Carries sync waits/updates onto the next
    kept instruction. Run before _legalize_waits."""
    import concourse.mybir as mybir
    removed = 0
    for f in nc.m.functions:
        for blk in f.blocks:
            out = []
            last_key = None
            pend_w, pend_u = [], []
            for ins in blk.instructions:
                nm = type(ins).__name__
                if nm == 'InstLdweights':
                    a = ins.ins[0]
                    key = (a.memref, a.offset, str(a.ap), str(a.dtype))
                    if key == last_key:
                        if ins.sync_info is not None:
                            pend_w.extend(ins.sync_info.on_wait)
                            pend_u.extend(ins.sync_info.on_update)
                        removed += 1
                        continue
                    last_key = key
                elif nm == 'InstMatmult':
                    pass          # keeps the loaded weights
                elif nm in ('InstNoOp', 'InstEventSemaphore'):
                    pass          # no effect on PE array state
                else:
                    last_key = None
                if pend_w or pend_u:
                    if ins.sync_info is None:
                        ins.sync_info = mybir.SyncInfo(on_wait=[], on_update=[])
                    ins.sync_info.on_wait = pend_w + list(ins.sync_info.on_wait)
                    ins.sync_info.on_update = (list(ins.sync_info.on_update)
                                               + pend_u)
                    pend_w, pend_u = [], []
                out.append(ins)
            blk.instructions[:] = out
    return removed


def _legalize_waits(nc, mybir, max_waits=1):
    """Split multi-wait instructions into single-wait NoOps.

    The walrus build here rejects >1 piggybacked sync wait per instruction
    (seen on S3_LW-lowered matmuls and Drains). Run after TileContext
    exit, before nc.finalize()."""
    ctr = 0
    n_split = 0
    for f in nc.m.functions:
        for blk in f.blocks:
            out = []
            for ins in blk.instructions:
                si = ins.sync_info
                if si is not None and len(si.on_wait) > max_waits:
                    waits = list(si.on_wait)
                    extra, keep = waits[:-max_waits], waits[-max_waits:]
                    for w in extra:
                        ctr += 1
                        nop = mybir.InstNoOp(name=f"waitnop-{ctr}")
                        nop.engine = ins.engine
                        nop.sync_info = mybir.SyncInfo(on_wait=[w], on_update=[])
                        out.append(nop)
                        n_split += 1
                    si.on_wait = keep
                out.append(ins)
            blk.instructions[:] = out
    return n_split


def _build_program():
    import concourse.bass as bass
    import concourse.mybir as mybir
    import concourse.tile as tile

    f32 = mybir.dt.float32
    mdt = mybir.dt.bfloat16 if MMDT == "bfloat16" else mybir.dt.float32r
    Alu = mybir.AluOpType
    Act = mybir.ActivationFunctionType

    nc = bass.Bass()

    # partition-major layouts: one ~1-2MB DMA per tensor/group
    xtr_d = nc.dram_tensor("xtr", [2, 128, 8 * XW], mdt, kind="ExternalInput")
    w_d = nc.dram_tensor("w", [M, 128, KA * 128], mdt, kind="ExternalInput")
    hr_d = nc.dram_tensor("hr", [GC, 128, 4 * HW], mdt, kind="ExternalInput")
    p_d = nc.dram_tensor("p", [M, 128, H], mdt, kind="ExternalInput")
    hpa_d = nc.dram_tensor("hpa", [128, M, B], f32, kind="ExternalInput")
    g_d = nc.dram_tensor("g", [128, M, B], f32, kind="ExternalInput")
    eb_d = nc.dram_tensor("eb", [RSH, B], mdt, kind="ExternalInput")
    bias_d = nc.dram_tensor("bias", [128, 2 * M + 1], f32, kind="ExternalInput")
    hn_d = nc.dram_tensor("hn", [128, M, B], f32, kind="ExternalOutput")
    pp_d = nc.dram_tensor("pp", [M, 128, HC, B], f32, kind="ExternalOutput")

    with tile.TileContext(nc) as tc:
        with (
            tc.tile_pool(name="consts", bufs=1) as consts,
            tc.tile_pool(name="xres", bufs=1) as xres,
            tc.tile_pool(name="wstream", bufs=4) as wstream,
            tc.tile_pool(name="cstream", bufs=2) as cstream,
            tc.tile_pool(name="pres", bufs=1) as pres,
            tc.tile_pool(name="sens", bufs=1) as senspool,
            tc.tile_pool(name="recs", bufs=1) as recpool,
            tc.tile_pool(name="hn", bufs=1) as hnpool,
            tc.tile_pool(name="tmp", bufs=2) as tmppool,
            tc.tile_pool(name="outb", bufs=2) as outpool,
        ):
            with (
                tc.tile_pool(name="acc", bufs=4, space="PSUM") as acc_pool,
                tc.tile_pool(name="psb", bufs=2, space="PSUM") as psb_pool,
                tc.tile_pool(name="psr", bufs=1, space="PSUM") as psr_pool,
            ):
                # ---- small constants ----
                eb_t = consts.tile([RSH, B], mdt, tag="eb")
                nc.sync.dma_start(eb_t[:], eb_d[:])
                bias_t = consts.tile([128, 2 * M + 1], f32, tag="bias")
                nc.sync.dma_start(bias_t[:], bias_d[:])

                # ---- phase C: rec accumulation over h_prev K-tiles ----
                rec_ps = [acc_pool.tile([128, B], f32, tag="acc",
                                        name=f"rec_ps{i}") for i in range(M)]
                p_t = pres.tile([128, M, H], mdt, tag="p")
                xt_t = xres.tile([128, KA, XW], mdt, tag="xt")
                hpa_t = consts.tile([128, M, B], f32, tag="hpa")
                g_t = consts.tile([128, M, B], f32, tag="g")
                for c in range(GC):
                    c_t = cstream.tile([128, 4, HW], mdt, tag="hr")
                    nc.sync.dma_start(c_t[:], hr_d[c])
                    for j in range(4):
                        k = 4 * c + j
                        ht = c_t[:, j, 0:B]
                        for m in range(M):
                            nc.tensor.matmul(
                                rec_ps[m][:],
                                c_t[:, j, B + 128 * m:B + 128 * (m + 1)],
                                ht, start=(k == 0), stop=(k == KC - 1))
                    # interleave x / P loads into phase C DMA slack
                    if c == 1:
                        nc.sync.dma_start(xt_t[:, 0:8, :], xtr_d[0])
                    elif c == 2:
                        nc.sync.dma_start(xt_t[:, 8:16, :], xtr_d[1])

                rec_sb = []
                for m in range(M):
                    r_sb = recpool.tile([128, B], f32, tag=f"rec{m}",
                                        name=f"rec_sb{m}")
                    nc.scalar.activation(r_sb[:], rec_ps[m][:], Act.Copy)
                    rec_sb.append(r_sb)

                # ---- phase A: router, then sensory m-outer ----
                w_ts = []
                for m in range(M):
                    w_t = wstream.tile([128, KA * 128], mdt, tag="w",
                                       name=f"w_t{m}")
                    nc.sync.dma_start(w_t[:], w_d[m])
                    w_ts.append(w_t)

                rg_ps = psr_pool.tile([RSH, B], f32, tag="rg")
                for k in range(KA):
                    nc.tensor.matmul(rg_ps[:], xt_t[:, k, B:XW],
                                     xt_t[:, k, 0:B],
                                     start=(k == 0), stop=(k == KA - 1))
                rg32 = tmppool.tile([RSH, B], f32, tag="rg32")
                nc.scalar.activation(rg32[:], rg_ps[:], Act.Sigmoid,
                                     bias=bias_t[0:RSH, 2 * M:2 * M + 1],
                                     scale=1.0)
                rg_r = tmppool.tile([RSH, B], mdt, tag="rgr")
                nc.vector.tensor_copy(rg_r[:], rg32[:])

                # remaining bulk loads ride phase A's DMA slack; P is
                # only needed from phase E so it queues last
                nc.sync.dma_start(hpa_t[:], hpa_d[:])
                nc.sync.dma_start(g_t[:], g_d[:])
                for kb in range(M):
                    nc.sync.dma_start(p_t[:, kb, :], p_d[kb])

                hn_sb = hnpool.tile([128, M, B], f32, tag="hnsb")
                hn_ts = []
                for m in range(M):
                    s_ps = acc_pool.tile([128, B], f32, tag="acc",
                                         name=f"s_ps{m}")
                    for k in range(KA):
                        nc.tensor.matmul(
                            s_ps[:], w_ts[m][:, 128 * k:128 * (k + 1)],
                            xt_t[:, k, 0:B],
                            start=(k == 0), stop=(k == KA - 1))
                    mask_ps = psb_pool.tile([128, B], f32, tag="mask",
                                            name=f"mask_ps{m}")
                    nc.tensor.matmul(mask_ps[:], eb_t[:, 128 * m:128 * (m + 1)],
                                     rg_r[:], start=True, stop=True)
                    mask_sb = tmppool.tile([128, B], f32, tag="masksb",
                                           name=f"mask_sb{m}")
                    nc.scalar.activation(mask_sb[:], mask_ps[:], Act.Copy)
                    s_sb = senspool.tile([128, B], f32, tag=f"sens{m}",
                                         name=f"sens_sb{m}")
                    # (x@W + bW) * mask
                    nc.vector.scalar_tensor_tensor(
                        s_sb[:], s_ps[:], bias_t[:, m:m + 1], mask_sb[:],
                        op0=Alu.add, op1=Alu.mult)
                    # ---- phase D chain for this m (overlaps next m's MMs)
                    tmp = tmppool.tile([128, B], f32, tag="dtmp", name=f"tmp{m}")
                    nc.vector.scalar_tensor_tensor(
                        tmp[:], rec_sb[m][:], bias_t[:, M + m:M + m + 1],
                        s_sb[:], op0=Alu.add, op1=Alu.add)
                    tgt = tmppool.tile([128, B], f32, tag="dtgt", name=f"tgt{m}")
                    nc.scalar.activation(tgt[:], tmp[:], Act.Tanh)
                    e_sb = tmppool.tile([128, B], f32, tag="de", name=f"e{m}")
                    nc.vector.tensor_mul(e_sb[:], tgt[:], g_t[:, m, :])
                    nc.vector.tensor_add(hn_sb[:, m, :], e_sb[:],
                                         hpa_t[:, m, :])
                    hn_r = hnpool.tile([128, B], mdt, tag=f"hnr{m}",
                                       name=f"hn_r{m}")
                    nc.vector.tensor_copy(hn_r[:], hn_sb[:, m, :])
                    hn_ts.append(hn_r)
                nc.sync.dma_start(hn_d[:], hn_sb[:])

            # ---- phase E: pred partial, batch-major out ----
            # out[bt rows (batch), hid] = sum_kb hn^T[kb, bt]^T @ P[kb]
            # stationary hn block reused by 8 moving P chunks (LDW dedup);
            # kb 0..2 matmuls overlap the tail of the m=3 update chain
            with tc.tile_pool(name="eps", bufs=8, space="PSUM") as eps:
                for bt in range(M):
                    pps = [eps.tile([128, B], f32, tag="ep",
                                    name=f"pp_{bt}_{hc}") for hc in range(HC)]
                    for kb in range(M):
                        for hc in range(HC):
                            nc.tensor.matmul(
                                pps[hc][:],
                                hn_ts[kb][:, 128 * bt:128 * (bt + 1)],
                                p_t[:, kb, B * hc:B * (hc + 1)],
                                start=(kb == 0), stop=(kb == M - 1),
                                skip_group_check=True)
                    pp_sb = outpool.tile([128, HC, B], f32, tag="ppsb",
                                         name=f"pp_sb{bt}")
                    for hc in range(HC):
                        nc.scalar.activation(pp_sb[:, hc, :], pps[hc][:],
                                             Act.Copy)
                        if hc % 2 == 1:
                            nc.sync.dma_start(pp_d[bt][:, hc - 1:hc + 1, :],
                                              pp_sb[:, hc - 1:hc + 1, :])

    _dedup_ldweights(nc)
    _legalize_waits(nc, mybir)
    nc.finalize()
    return nc


def _get_program():
    global _prog
    if _prog is None:
        _prog = _build_program()
    return _prog


def _densify(rows, cols, vals, n_rows, n_cols):
    flat = rows.astype(np.int64) * n_cols + cols.astype(np.int64)
    dense = np.bincount(flat, weights=vals.astype(np.float64),
                        minlength=n_rows * n_cols)
    return dense.astype(np.float32).reshape(n_rows, n_cols)


def _group_pmajor(tiles, width, group=4):
    """[K,128,width] -> [K/group, 128, group*width] with `group` K-tiles
    contiguous per partition row (large per-partition runs per DMA)."""
    k = tiles.shape[0]
    return np.ascontiguousarray(
        tiles.reshape(k // group, group, 128, width)
        .transpose(0, 2, 1, 3).reshape(k // group, 128, group * width))


def kernel(x, h_prev, gate, W_vals, W_bias, R_vals, R_bias, P_vals, P_bias,
           router_w, router_b, tau, W_rows, W_cols, R_rows, R_cols,
           P_rows, P_cols):
    from concourse.bass_utils import run_bass_kernel_spmd

    if MMDT == "bfloat16":
        import ml_dtypes
        mmdt = ml_dtypes.bfloat16
    else:
        mmdt = np.float32

    x = np.asarray(x, np.float32)
    h_prev = np.asarray(h_prev, np.float32)
    gate = np.asarray(gate, np.float32)

    Wd = _densify(np.asarray(W_rows), np.asarray(W_cols), np.asarray(W_vals), IN, H)
    Rd = _densify(np.asarray(R_rows), np.asarray(R_cols), np.asarray(R_vals), H, H)
    Pd = _densify(np.asarray(P_rows), np.asarray(P_cols), np.asarray(P_vals), H, H)

    XT = np.ascontiguousarray(x.T).reshape(KA, 128, B)
    HpT = np.ascontiguousarray(h_prev.T).reshape(KC, 128, B)
    rwT = np.ascontiguousarray(np.asarray(router_w, np.float32).T)  # [2048, 64]
    dt_tau = (DT / np.asarray(tau, np.float32))                     # [4096]
    gate1 = gate.reshape(B)

    ebmat = np.zeros((RSH, B), np.float32)
    for j in range(RSH):
        ebmat[j, 64 * j:64 * (j + 1)] = 1.0
    ebmat = ebmat.astype(mmdt)

    in_maps = []
    for c in range(NCORES):
        sh = slice(SH * c, SH * (c + 1))
        w_slab = np.ascontiguousarray(Wd[:, sh]).reshape(KA, 128, SH)
        # per-m partition-major: w2[m, p, 128*k+q] = W[128k+p, 128m+q]
        w2 = np.ascontiguousarray(
            w_slab.reshape(KA, 128, M, 128).transpose(2, 1, 0, 3)
            .reshape(M, 128, KA * 128)).astype(mmdt)
        r_slab = np.ascontiguousarray(Rd[:, sh]).reshape(KC, 128, SH)
        p_slab = np.ascontiguousarray(Pd[sh, :]).reshape(M, 128, H)
        rw_slab = np.ascontiguousarray(
            rwT[:, RSH * c:RSH * (c + 1)]).reshape(KA, 128, RSH)
        xtr = np.concatenate([XT, rw_slab], axis=2).astype(mmdt)    # [16,128,520]
        hr = np.concatenate([HpT, r_slab], axis=2).astype(mmdt)
        g_full = np.outer(dt_tau[sh], gate1).astype(np.float32)     # [SH, B]
        hp_sh = np.ascontiguousarray(h_prev.T[sh])                  # [SH, B]
        hpa_full = (hp_sh - g_full * hp_sh).astype(np.float32)      # hp*(1-g)
        hpa = np.ascontiguousarray(
            hpa_full.reshape(M, 128, B).transpose(1, 0, 2))
        g = np.ascontiguousarray(
            g_full.reshape(M, 128, B).transpose(1, 0, 2))
        bias = np.zeros((128, 2 * M + 1), np.float32)
        bias[:, 0:M] = np.asarray(W_bias, np.float32)[sh].reshape(M, 128).T
        bias[:, M:2 * M] = np.asarray(R_bias, np.float32)[sh].reshape(M, 128).T
        bias[0:RSH, 2 * M] = np.asarray(router_b, np.float32)[RSH * c:RSH * (c + 1)]
        in_maps.append({
            "xtr": _group_pmajor(xtr, XW, group=8),
            "w": w2,
            "hr": _group_pmajor(hr, HW, group=4),
            "p": p_slab.astype(mmdt),
            "hpa": hpa,
            "g": g,
            "eb": ebmat,
            "bias": bias,
        })

    nc = _get_program()
    res = run_bass_kernel_spmd(nc, in_maps, list(range(NCORES)))

    h_new = np.empty((B, H), np.float32)
    pred = np.zeros((B, H), np.float64)
    for c in range(NCORES):
        sh = slice(SH * c, SH * (c + 1))
        h_new[:, sh] = res.results[c]["hn"].transpose(1, 0, 2).reshape(SH, B).T
        pred += res.results[c]["pp"].reshape(M * 128, H)
    pred = pred.astype(np.float32) + np.asarray(P_bias, np.float32)
    return (h_new, pred)
